# revision 1
# baseline (speedup 1.0000x reference)
"""MultiHeadSelectiveAttention TRN2 kernel: FULL inputs -> FULL output.

Shards batch (B=8) across 8 NeuronCores (data-parallel, one batch element
per core). Per batch b, using the value-head-dim-1 collapse:
    v  = x Wv + bv                      [L, H]
    xv = x^T v                          [D, H]
    ktv = blockdiag_mask(Wk^T xv + bk (x) sum_l v)      [D, H]
    U  = Wq ktv ;  c[h] = bq . ktv[:, h]
    out = sigmoid((x U + c)/8)^T * mask                 [H, L]
identical in exact arithmetic to the reference attention. Big matmuls run
in float32r (PE rounds operands to 12-bit-mantissa RNE); stationary
operands are exact hi/lo packed pairs; Wk/WqT movers are hi/lo paired.
"""
import sys, os
sys.path.insert(0, '/opt/trn_rl_repo')
import numpy as np


import sys
sys.path.insert(0, '/opt/trn_rl_repo')
from contextlib import ExitStack
import numpy as np
import concourse.bass as bass
import concourse.tile as tile
import concourse.mybir as mybir
from concourse.tile import ScopedClock
from concourse.masks import make_identity

f32 = mybir.dt.float32
f32r = mybir.dt.float32r
Sigmoid = mybir.ActivationFunctionType.Sigmoid

L, D, H = 4096, 1024, 16
NLT, NDT = L // 128, D // 128   # 32, 8
BLK = 4                          # l-tiles per block
NBLK = NLT // BLK                # 8

_wait_fix_counter = [0]
SPLIT_WAITS = [True]

def _split_multi_waits(nc):
    for f in nc.m.functions:
        for bb in f.blocks:
            new_insts = []
            for inst in bb.instructions:
                si = getattr(inst, 'sync_info', None)
                if si is not None and len(si.on_wait) > 1:
                    waits = list(si.on_wait)
                    for w in waits[:-1]:
                        _wait_fix_counter[0] += 1
                        nop = mybir.InstNoOp(
                            name=f"waitfix-{_wait_fix_counter[0]}",
                            engine=inst.engine, opcode="NoOp", ins=[], outs=[],
                            sync_info=mybir.SyncInfo(on_wait=[w], on_update=[]),
                        )
                        new_insts.append(nop)
                    inst.sync_info = mybir.SyncInfo(
                        on_wait=[waits[-1]], on_update=list(si.on_update))
                new_insts.append(inst)
            bb.instructions[:] = new_insts

def _drain_and_barrier_split(self, tick_clock, wait_clock):
    nc = self.nc
    probe = nc.sync.nop()
    wait_clock.add_sem_waits(probe.ins, ScopedClock({None: tick_clock.global_clock}))
    nc.sync.drain()
    nc.all_engine_barrier()
    assert self.sems is not None
    popped = nc._tile_sem_poison_stack.pop()
    assert popped is self._sem_poison
    nc.clear_and_free_semaphores(list(self.sems.allocated().values()))
    nc.all_engine_barrier()
    if SPLIT_WAITS[0]:
        _split_multi_waits(nc)

tile.TileContext._drain_and_barrier = _drain_and_barrier_split


def build(dump=()):
    nc = bass.Bass(trn_type="TRN2")
    x = nc.dram_tensor("x", [L, D], f32r, kind="ExternalInput")
    wq = nc.dram_tensor("wq", [D, D], f32, kind="ExternalInput")
    wk = nc.dram_tensor("wk", [D, D], f32r, kind="ExternalInput")
    wv = nc.dram_tensor("wv", [D, H], f32, kind="ExternalInput")
    bq = nc.dram_tensor("bq", [D, 1], f32r, kind="ExternalInput")
    bk = nc.dram_tensor("bk", [H, D], f32, kind="ExternalInput")
    bv = nc.dram_tensor("bv", [128, H], f32, kind="ExternalInput")
    mk = nc.dram_tensor("mk", [H, L], f32, kind="ExternalInput")
    bvc = nc.dram_tensor("bvc", [H, 1], f32, kind="ExternalInput")
    out = nc.dram_tensor("out", [H, L], f32, kind="ExternalOutput")
    dumps = {}
    if "ut" in dump:
        dump = tuple(dump) + ("c",)
    for name, shape in [("v", [NLT * 128, H]), ("xvt", [H, D]),
                        ("ktvbdt", [H, D]), ("ut", [H, D]), ("c", [H, 1])]:
        if name in dump:
            dumps[name] = nc.dram_tensor("d_" + name, shape, f32, kind="ExternalOutput")

    with ExitStack() as ctx:
        tc = ctx.enter_context(tile.TileContext(nc))
        konst = ctx.enter_context(tc.tile_pool(name="konst", bufs=1))
        xtrp = ctx.enter_context(tc.tile_pool(name="xtr", bufs=1))
        pers = ctx.enter_context(tc.tile_pool(name="pers", bufs=1))
        ps_xv = ctx.enter_context(tc.tile_pool(name="ps_xv", bufs=1, space="PSUM"))

        # ---------------- constants ----------------
        ident = konst.tile([128, 128], f32)
        make_identity(nc, ident[:])
        identr = konst.tile([128, 128], f32r)
        nc.vector.tensor_copy(identr[:], ident[:])
        ident_r = identr[:]
        bvt = konst.tile([128, H], f32)
        nc.sync.dma_start(bvt[:], bv[:, :])
        wvp = []
        for d in range(NDT):
            t = konst.tile([128, H], f32, tag=f"wvf{d}")
            nc.sync.dma_start(t[:], wv[128 * d:128 * d + 128, :])
            p = konst.tile([128, 3 * H], f32r, tag=f"wvp{d}")
            nc.vector.memset(p[:, H:2 * H].bitcast(f32), 0.0)
            nc.scalar.copy(p[:, 0:H], t[:])
            nc.vector.tensor_sub(p[:, 2 * H:3 * H], t[:], p[:, 0:H].bitcast(f32))
            wvp.append(p)
        bqc = []
        for d in range(NDT):
            t = konst.tile([128, 2], f32r, tag=f"bqc{d}")
            nc.vector.memset(t[:].bitcast(f32), 0.0)
            nc.sync.dma_start(t[:, 0:1], bq[128 * d:128 * d + 128, :])
            bqc.append(t)
        xtr = [xtrp.tile([128, L], f32r, name=f"xtr{d}", tag=f"xtr{d}") for d in range(NDT)]
        xvt_ps = [ps_xv.tile([48, 512], f32, name=f"xv{c}", tag=f"xv{c}") for c in range(2)]
        n_xv = [0]
        svps = []

        # ---------------- PHASE A ----------------
        with tc.tile_pool(name="phA", bufs=2) as sbA, \
             tc.tile_pool(name="xnatp", bufs=3) as xnatp, \
             tc.tile_pool(name="vpbp", bufs=4) as vpbp, \
             tc.tile_pool(name="ps_tr", bufs=3, space="PSUM") as ps_tr, \
             tc.tile_pool(name="ps_v", bufs=2, space="PSUM") as ps_v, \
             tc.tile_pool(name="ps_f", bufs=1, space="PSUM") as ps_f:
            for blk in range(NBLK):
                lts = [BLK * blk + j for j in range(BLK)]
                xblk = xnatp.tile([128, BLK * D], f32r, tag="xnat")
                nc.sync.dma_start(
                    xblk[:].rearrange("p (j d) -> p j d", j=BLK),
                    x[512 * blk:512 * blk + 512, :]
                    .rearrange("(j p) d -> p j d", p=128))
                xnat = [xblk[:, D * j:D * (j + 1)] for j in range(BLK)]
                for d in range(NDT):
                    ps = ps_tr.tile([128, 512], f32r, tag="tr")
                    for j in range(BLK):
                        nc.tensor.matmul(
                            ps[:, 128 * j:128 * j + 128],
                            xnat[j][:, 128 * d:128 * d + 128],
                            ident_r,
                            start=True, stop=True, is_transpose=True,
                            skip_group_check=True)
                    if d % 3 == 0:
                        nc.scalar.copy(xtr[d][:, 512 * blk:512 * blk + 512], ps[:])
                    else:
                        nc.vector.tensor_copy(xtr[d][:, 512 * blk:512 * blk + 512], ps[:])
                # P1: vT for block, accumulate over d
                psv = ps_v.tile([48, 512], f32, tag="v")
                for d in range(NDT):
                    nc.tensor.matmul(
                        psv[:], wvp[d][:], xtr[d][:, 512 * blk:512 * blk + 512],
                        start=(d == 0), stop=(d == NDT - 1))
                vts = sbA.tile([48, 512], f32, tag="vts")
                svp = sbA.tile([48, 1], f32, name="svp", tag=f"svp{blk}", bufs=1)
                nc.scalar.activation(vts[:], psv[:],
                                     mybir.ActivationFunctionType.Copy,
                                     accum_out=svp[:])
                svps.append(svp)
                # fold-transpose to v-natural groups [128, 32] per l-tile
                psf = ps_f.tile([128, 192], f32, tag="vf")
                for j in range(BLK):
                    nc.tensor.matmul(
                        psf[:, 48 * j:48 * j + 48],
                        vts[:, 128 * j:128 * j + 128],
                        ident[0:48, 0:48],
                        start=True, stop=True, is_transpose=True,
                        skip_group_check=True)
                # vsum[128, BLK*16] = hi-stat + lo-stat + bv
                vsum = sbA.tile([128, BLK * 16], f32, tag="vsum")
                psf_f = psf[:].rearrange("p (j x) -> p j x", j=BLK)
                vs3 = vsum[:].rearrange("p (j h) -> p j h", j=BLK)
                nc.scalar.copy(vs3, psf_f[:, :, 0:16])
                nc.vector.tensor_add(vs3, vs3, psf_f[:, :, 32:48])
                nc.vector.tensor_add(
                    vs3, vs3, bvt[:].unsqueeze(1).broadcast_to([128, BLK, H]))
                if "v" in dump:
                    for j in range(BLK):
                        nc.gpsimd.dma_start(
                            dumps["v"][128 * lts[j]:128 * lts[j] + 128, :],
                            vsum[:, 16 * j:16 * j + 16])
                vpb = vpbp.tile([128, BLK * 48], f32r, tag="vpb")
                vp4 = vpb[:].rearrange("p (j x) -> p j x", j=BLK)
                nc.vector.memset(vp4[:, :, 16:32].bitcast(f32), 0.0)
                nc.scalar.copy(vp4[:, :, 0:16], vs3)
                nc.vector.tensor_sub(
                    vp4[:, :, 32:48], vs3, vp4[:, :, 0:16].bitcast(f32))
                # P2 + sv
                for j in range(BLK):
                    n_xv[0] += 1
                    for c in range(2):
                        nc.tensor.matmul(
                            xvt_ps[c][:], vpb[:, 48 * j:48 * j + 48],
                            xnat[j][:, 512 * c:512 * c + 512],
                            start=(n_xv[0] == 1), stop=(n_xv[0] == NLT))

        xvt = pers.tile([H, D], f32, tag="xvt")
        for c in range(2):
            sl = xvt[:, 512 * c:512 * c + 512]
            nc.scalar.copy(sl, xvt_ps[c][0:16, :])
            nc.vector.tensor_add(sl, sl, xvt_ps[c][32:48, :])
        svacc = pers.tile([48, 1], f32, tag="svacc")
        nc.vector.tensor_add(svacc[:], svps[0][:], svps[1][:])
        for b in range(2, NBLK):
            nc.vector.tensor_add(svacc[:], svacc[:], svps[b][:])
        sv = pers.tile([H, 1], f32, tag="sv")
        svlo = pers.tile([H, 1], f32, tag="svlo")
        nc.scalar.copy(svlo[:], svacc[32:48, :])
        nc.vector.tensor_add(sv[:], svacc[0:16, :], svlo[:])
        bvcol = pers.tile([H, 1], f32, tag="bvcol")
        nc.sync.dma_start(bvcol[:], bvc[:, :])
        nc.scalar.mul(bvcol[:], bvcol[:], float(L))
        nc.vector.tensor_add(sv[:], sv[:], bvcol[:])
        if "xvt" in dump:
            nc.gpsimd.dma_start(dumps["xvt"][:, :], xvt[:])

        # ---------------- PHASE B ----------------
        with tc.tile_pool(name="phB", bufs=2) as sbB:
            bkt = sbB.tile([H, D], f32, tag="big4k", bufs=1)
            nc.sync.dma_start(bkt[:], bk[:, :])
            bdmt = []
            for k in range(NDT):
                m = sbB.tile([128, H], f32, name=f"bdmt{k}", tag=f"bdmt{k}", bufs=1)
                nc.vector.memset(m[:], 0.0)
                nc.vector.memset(m[0:64, 2 * k:2 * k + 1], 1.0)
                nc.vector.memset(m[64:128, 2 * k + 1:2 * k + 2], 1.0)
                bdmt.append(m)
            # xv pairs [128, 32] f32r per d-tile (transpose xvt)
            xvp = []
            with tc.tile_pool(name="ps_m1", bufs=2, space="PSUM") as ps_m:
              for d in range(NDT):
                psm = ps_m.tile([128, 16], f32, name="psm", tag="sm")
                nc.tensor.matmul(
                    psm[:], xvt[0:16, 128 * d:128 * d + 128], ident[0:16, 0:16],
                    start=True, stop=True, is_transpose=True, skip_group_check=True)
                p = sbB.tile([128, 48], f32r, name=f"xvp{d}", tag=f"xvp{d}", bufs=1)
                nc.vector.memset(p[:, 16:32].bitcast(f32), 0.0)
                nc.scalar.copy(p[:, 0:16], psm[:])
                nc.vector.tensor_sub(p[:, 32:48], psm[:], p[:, 0:16].bitcast(f32))
                xvp.append(p)
            # step 3: KTVfullT = xv^T Wk   (Wk JIT-streamed)
            ps_s3 = ctx.enter_context(tc.tile_pool(name="ps_s3", bufs=1, space="PSUM"))
            ps3 = [ps_s3.tile([48, 512], f32, name=f"s3{c}", tag=f"s3{c}") for c in range(2)]
            with tc.tile_pool(name="wkjit", bufs=3) as wkjit:
                for k in range(NDT):
                    wkt = wkjit.tile([128, D], f32, name="wkt", tag="wkt")
                    nc.sync.dma_start(wkt[:], wk[128 * k:128 * k + 128, :].bitcast(f32))
                    wkhi = wkjit.tile([128, D], f32r, name="wkhi", tag="wkhi")
                    nc.scalar.copy(wkhi[:], wkt[:])
                    wklo = wkjit.tile([128, D], f32r, name="wklo", tag="wklo")
                    nc.vector.tensor_sub(wklo[:], wkt[:], wkhi[:].bitcast(f32))
                    for c in range(2):
                        nc.tensor.matmul(
                            ps3[c][:], xvp[k][:], wkhi[:, 512 * c:512 * c + 512],
                            start=(k == 0), stop=False)
                        nc.tensor.matmul(
                            ps3[c][:], xvp[k][:], wklo[:, 512 * c:512 * c + 512],
                            start=False, stop=(k == NDT - 1))
            # ktvbdt = (halves-sum + bk*sv) * bdm
            ktvbdt = sbB.tile([H, D], f32, tag="ktvbdt", bufs=1)
            nc.scalar.activation(
                ktvbdt[:], bkt[:],
                mybir.ActivationFunctionType.Copy, scale=sv[:])
            for c in range(2):
                sl = ktvbdt[:, 512 * c:512 * c + 512]
                nc.vector.tensor_add(sl, sl, ps3[c][0:16, :])
                nc.vector.tensor_add(sl, sl, ps3[c][32:48, :])
            if "ktvbdt" in dump:
                nc.gpsimd.dma_start(dumps["ktvbdt"][:, :], ktvbdt[:])
            # ktv pairs per p-tile + c accumulation
            ktvp = []
            with tc.tile_pool(name="ps_m2", bufs=2, space="PSUM") as ps_m:
              for k in range(NDT):
                psm = ps_m.tile([128, 16], f32, name="psm", tag="sm")
                nc.tensor.matmul(
                    psm[:], ktvbdt[0:16, 128 * k:128 * k + 128], ident[0:16, 0:16],
                    start=True, stop=True, is_transpose=True, skip_group_check=True)
                p = sbB.tile([128, 48], f32r, name=f"ktvp{k}", tag=f"ktvp{k}", bufs=1)
                nc.vector.tensor_mul(psm[:], psm[:], bdmt[k][:])
                nc.vector.memset(p[:, 16:32].bitcast(f32), 0.0)
                nc.scalar.copy(p[:, 0:16], psm[:])
                nc.vector.tensor_sub(p[:, 32:48], psm[:], p[:, 0:16].bitcast(f32))
                ktvp.append(p)
            with tc.tile_pool(name="ps_c", bufs=1, space="PSUM") as ps_c:
                psc = ps_c.tile([48, 2], f32, tag="c", bufs=1)
                for k in range(NDT):
                    nc.tensor.matmul(
                        psc[:], ktvp[k][:], bqc[k][:],
                        start=(k == 0), stop=(k == NDT - 1))
                cdiv8 = sbB.tile([H, 1], f32, tag="cdiv8", bufs=1)
                nc.scalar.copy(cdiv8[:], psc[0:16, 0:1])
                nc.vector.tensor_add(cdiv8[:], cdiv8[:], psc[32:48, 0:1])
                nc.scalar.mul(cdiv8[:], cdiv8[:], 0.125)
            # step 4: UT accumulation with WqT JIT (transpose Wq per p-tile b)
            ps_s4 = ctx.enter_context(tc.tile_pool(name="ps_s4", bufs=1, space="PSUM"))
            ps4 = [ps_s4.tile([48, 512], f32, name=f"s4{c}", tag=f"s4{c}") for c in range(2)]
            with tc.tile_pool(name="wqcp", bufs=2) as wqcp, \
                 tc.tile_pool(name="wqtjit", bufs=2) as wqtjit, \
                 tc.tile_pool(name="ps_q", bufs=2, space="PSUM") as ps_q:
                for b in range(NDT):
                    wc = wqcp.tile([128, D], f32, name="wqc", tag="wqc")
                    nc.sync.dma_start(
                        wc[:].rearrange("p (k j) -> p k j", k=NDT),
                        wq[:, 128 * b:128 * b + 128]
                        .rearrange("(k p) j -> p k j", p=128))
                    wt = wqtjit.tile([128, D], f32r, name="wqt", tag="wqt")
                    wtlo = wqtjit.tile([128, D], f32r, name="wqtlo", tag="wqtlo")
                    for half in range(2):
                        psq = ps_q.tile([128, 512], f32, name="psq", tag="q")
                        for kk in range(4):
                            k = 4 * half + kk
                            nc.tensor.matmul(
                                psq[:, 128 * kk:128 * kk + 128],
                                wc[:, 128 * k:128 * k + 128], ident,
                                start=True, stop=True, is_transpose=True,
                                skip_group_check=True)
                        nc.scalar.copy(wt[:, 512 * half:512 * half + 512], psq[:])
                        nc.vector.tensor_sub(
                            wtlo[:, 512 * half:512 * half + 512], psq[:],
                            wt[:, 512 * half:512 * half + 512].bitcast(f32))
                    for c in range(2):
                        nc.tensor.matmul(
                            ps4[c][:], ktvp[b][:], wt[:, 512 * c:512 * c + 512],
                            start=(b == 0), stop=False)
                        nc.tensor.matmul(
                            ps4[c][:], ktvp[b][:], wtlo[:, 512 * c:512 * c + 512],
                            start=False, stop=(b == NDT - 1))
            ut = sbB.tile([H, D], f32, name="ut", tag="big4k", bufs=1)
            for c in range(2):
                sl = ut[:, 512 * c:512 * c + 512]
                nc.scalar.copy(sl, ps4[c][0:16, :])
                nc.vector.tensor_add(sl, sl, ps4[c][32:48, :])
            if "ut" in dump:
                nc.gpsimd.dma_start(dumps["ut"][:, :], ut[:])
                nc.gpsimd.dma_start(dumps["c"][:, :], cdiv8[:])
            # U pairs per d-tile
            upr = []
            with tc.tile_pool(name="ps_m3", bufs=2, space="PSUM") as ps_m:
              for d in range(NDT):
                psm = ps_m.tile([128, 16], f32, name="psm", tag="sm")
                nc.tensor.matmul(
                    psm[:], ut[0:16, 128 * d:128 * d + 128], ident[0:16, 0:16],
                    start=True, stop=True, is_transpose=True, skip_group_check=True)
                p = sbB.tile([128, 48], f32r, name=f"upr{d}", tag=f"upr{d}", bufs=1)
                nc.vector.memset(p[:, 16:32].bitcast(f32), 0.0)
                nc.scalar.copy(p[:, 0:16], psm[:])
                nc.vector.tensor_sub(p[:, 32:48], psm[:], p[:, 0:16].bitcast(f32))
                upr.append(p)
            # P5: zT chunks + sigmoid + mask + store
            ps_5 = ctx.enter_context(tc.tile_pool(name="ps_5", bufs=2, space="PSUM"))
            for ch in range(8):
                ps5 = ps_5.tile([48, 512], f32, name="ps5", tag="s5")
                for d in range(NDT):
                    nc.tensor.matmul(
                        ps5[:], upr[d][:], xtr[d][:, 512 * ch:512 * ch + 512],
                        start=(d == 0), stop=(d == NDT - 1))
                mkc = sbB.tile([H, 512], f32, name="mkc", tag="mkc")
                nc.sync.dma_start(mkc[:], mk[:, 512 * ch:512 * ch + 512])
                zs = sbB.tile([H, 512], f32, name="zs", tag="zs")
                nc.scalar.copy(zs[:], ps5[0:16, :])
                nc.vector.tensor_add(zs[:], zs[:], ps5[32:48, :])
                sg = sbB.tile([H, 512], f32, name="sg", tag="sg")
                nc.scalar.activation(sg[:], zs[:], Sigmoid, bias=cdiv8[:], scale=0.125)
                nc.vector.tensor_mul(sg[:], sg[:], mkc[:])
                nc.scalar.dma_start(out[:, 512 * ch:512 * ch + 512], sg[:])
    return nc, dumps


def ref_numpy(x, wq, wk, wv, bq, bk, bv):
    """f64 reference of the decomposed math for per-stage validation."""
    x64 = x.astype(np.float64)
    v = x64 @ wv.astype(np.float64) + bv.astype(np.float64)   # [L, H]
    xv = x64.T @ v                                            # [D, H]
    ktvfull = wk.astype(np.float64).T @ xv                    # [D(hd), H]
    sv = v.sum(axis=0)                                        # [H]
    ktvfull = ktvfull + np.outer(bk.astype(np.float64), sv)
    bd = np.zeros((D, H))
    for h in range(H):
        bd[64 * h:64 * h + 64, h] = 1.0
    ktvbd = ktvfull * bd
    u = wq.astype(np.float64) @ ktvbd                         # [D, H]
    c = bq.astype(np.float64) @ ktvbd                         # [H]
    z = (x64 @ u + c) / 8.0                                   # [L, H]
    p = 1.0 / (1.0 + np.exp(-z))
    return dict(v=v, xvt=xv.T, ktvbdt=ktvbd.T, ut=u.T, c=c / 8.0, out=p.T)




B = 8
_cache = {}

def _get_nc():
    if "nc" not in _cache:
        _cache["nc"] = build()[0]
    return _cache["nc"]


def kernel(x, mask, Wq, bq, Wk, bk, Wv, bv):
    from concourse.bass_utils import run_bass_kernel_spmd
    x = np.asarray(x, dtype=np.float32)
    mask_f = np.asarray(mask).astype(np.float32)
    Wq = np.ascontiguousarray(np.asarray(Wq, dtype=np.float32))
    Wk = np.ascontiguousarray(np.asarray(Wk, dtype=np.float32))
    Wv = np.ascontiguousarray(np.asarray(Wv, dtype=np.float32))
    bq = np.asarray(bq, dtype=np.float32)
    bk = np.asarray(bk, dtype=np.float32)
    bv = np.asarray(bv, dtype=np.float32)
    nc = _get_nc()
    bk2 = np.ascontiguousarray(np.broadcast_to(bk[None, :], (H, D)))
    bv2 = np.ascontiguousarray(np.broadcast_to(bv[None, :], (128, H)))
    bqc_ = np.ascontiguousarray(bq.reshape(D, 1))
    bvc_ = np.ascontiguousarray(bv.reshape(H, 1))
    in_maps = []
    for b in range(B):
        in_maps.append({
            "x": np.ascontiguousarray(x[b]),
            "wq": Wq, "wk": Wk, "wv": Wv,
            "bq": bqc_, "bk": bk2, "bv": bv2, "bvc": bvc_,
            "mk": np.ascontiguousarray(
                np.broadcast_to(mask_f[b][None, :], (H, L))),
        })
    res = run_bass_kernel_spmd(nc, in_maps, core_ids=list(range(B)))
    out = np.stack([res.results[b]["out"] for b in range(B)], axis=0)
    return out.astype(np.float32)



# revision 5
# speedup vs baseline: 1.1992x; 1.1992x over previous
"""MultiHeadSelectiveAttention TRN2 kernel: FULL inputs -> FULL output.

Shards batch (B=8) across 8 NeuronCores (data-parallel, one batch element
per core). Per batch b, using the value-head-dim-1 collapse:
    v   = x Wv + bv                        [L, H]
    xv  = x^T v                            [D, H]
    ktv = blockdiag_mask(Wk^T xv + bk (x) sum_l v)   [D, H]
    U   = Wq ktv ;  c[h] = bq . ktv[:, h]
    out = sigmoid((x U + c)/8)^T * mask    [H, L]
identical in exact arithmetic to the reference attention.

v2: single-rounded math (no hi/lo pairs), x path in bf16 (cast during DMA,
FWL-fast PE transposes), weights in f32r, all constants batched into one
staging DMA, mask applied on host (it is a no-op for all-ones masks).
"""
import sys
sys.path.insert(0, '/opt/trn_rl_repo')
from contextlib import ExitStack
import numpy as np
import concourse.bass as bass
import concourse.tile as tile
import concourse.mybir as mybir
from concourse.tile import ScopedClock
from concourse.masks import make_identity

f32 = mybir.dt.float32
f32r = mybir.dt.float32r
bf16 = mybir.dt.bfloat16
Copy = mybir.ActivationFunctionType.Copy
Sigmoid = mybir.ActivationFunctionType.Sigmoid

L, D, H = 4096, 1024, 16
NLT, NDT = L // 128, D // 128   # 32, 8
BLK = 4                          # l-tiles per block
NBLK = NLT // BLK                # 8

_wait_fix_counter = [0]
SPLIT_WAITS = [True]

def _split_multi_waits(nc):
    for f in nc.m.functions:
        for bb in f.blocks:
            new_insts = []
            for inst in bb.instructions:
                si = getattr(inst, 'sync_info', None)
                if si is not None and len(si.on_wait) > 1:
                    waits = list(si.on_wait)
                    for w in waits[:-1]:
                        _wait_fix_counter[0] += 1
                        nop = mybir.InstNoOp(
                            name=f"waitfix-{_wait_fix_counter[0]}",
                            engine=inst.engine, opcode="NoOp", ins=[], outs=[],
                            sync_info=mybir.SyncInfo(on_wait=[w], on_update=[]),
                        )
                        new_insts.append(nop)
                    inst.sync_info = mybir.SyncInfo(
                        on_wait=[waits[-1]], on_update=list(si.on_update))
                new_insts.append(inst)
            bb.instructions[:] = new_insts

def _drain_and_barrier_split(self, tick_clock, wait_clock):
    nc = self.nc
    probe = nc.sync.nop()
    wait_clock.add_sem_waits(probe.ins, ScopedClock({None: tick_clock.global_clock}))
    nc.sync.drain()
    nc.all_engine_barrier()
    assert self.sems is not None
    popped = nc._tile_sem_poison_stack.pop()
    assert popped is self._sem_poison
    nc.clear_and_free_semaphores(list(self.sems.allocated().values()))
    nc.all_engine_barrier()
    if SPLIT_WAITS[0]:
        _split_multi_waits(nc)

tile.TileContext._drain_and_barrier = _drain_and_barrier_split


def build(xbf16=True):
    nc = bass.Bass(trn_type="TRN2")
    x = nc.dram_tensor("x", [L, D], f32 if xbf16 else f32r,
                       kind="ExternalInput")
    wk = nc.dram_tensor("wk", [D, D], f32r, kind="ExternalInput")
    wq = nc.dram_tensor("wq", [D, D], f32r, kind="ExternalInput")
    # cst packs: cols 0:128 Wv d-tiles; 128:144 bq (dup pairs);
    # 144:160 bv bcast; col 160 rows 0:16 bv column
    cst = nc.dram_tensor("cst", [128, 176], f32, kind="ExternalInput")
    bkb = nc.dram_tensor("bkb", [H, D], f32, kind="ExternalInput")
    out = nc.dram_tensor("out", [H, L], f32, kind="ExternalOutput")

    xdt = bf16 if xbf16 else f32r

    with ExitStack() as ctx:
        tc = ctx.enter_context(tile.TileContext(nc))
        konst = ctx.enter_context(tc.tile_pool(name="konst", bufs=1))
        xtrp = ctx.enter_context(tc.tile_pool(name="xtr", bufs=1))
        pers = ctx.enter_context(tc.tile_pool(name="pers", bufs=1))
        wkp = ctx.enter_context(tc.tile_pool(name="wkp", bufs=1))
        ps_xv = ctx.enter_context(tc.tile_pool(name="ps_xv", bufs=1, space="PSUM"))

        # ---------------- constants ----------------
        ident = konst.tile([128, 128], f32)
        make_identity(nc, ident[:])
        identr = konst.tile([128, 128], f32r)
        nc.vector.tensor_copy(identr[:], ident[:])
        identx = konst.tile([128, 128], xdt)
        nc.vector.tensor_copy(identx[:], ident[:])
        cstt = konst.tile([128, 176], f32)
        nc.sync.dma_start(cstt[:], cst[:, :])
        # wk tiles: front-load in bf16 mode (SBUF allows), stream otherwise
        wk_bufs = 1 if xbf16 else None
        if xbf16:
            wkt = [wkp.tile([128, D], f32r, name=f"wk{k}", tag=f"wk{k}")
                   for k in range(NDT)]
            for k in range(NDT):
                nc.sync.dma_start(wkt[k][:], wk[128 * k:128 * k + 128, :])
        bkt = pers.tile([H, D], f32)
        nc.sync.dma_start(bkt[:], bkb[:, :])
        wvx = konst.tile([128, 128], xdt)
        nc.vector.tensor_copy(wvx[:], cstt[:, 0:128])
        wv_d = [wvx[:, 16 * k:16 * k + 16] for k in range(NDT)]
        bqc = konst.tile([128, 16], f32r)
        nc.vector.tensor_copy(bqc[:], cstt[:, 128:144])
        bvtb = konst.tile([128, H], xdt)
        nc.vector.tensor_copy(bvtb[:], cstt[:, 144:160])

        xtr = [xtrp.tile([128, L], xdt, name=f"xtr{d}", tag=f"xtr{d}")
               for d in range(NDT)]
        xv_ps = [ps_xv.tile([16, 512], f32, name=f"xv{c}", tag=f"xv{c}")
                 for c in range(2)]
        nxv = [0]
        svps = []

        # ---------------- PHASE A ----------------
        with tc.tile_pool(name="xnatp", bufs=3) as xnatp, \
             tc.tile_pool(name="sbA", bufs=2) as sbA, \
             tc.tile_pool(name="vnp", bufs=3) as vnp, \
             tc.tile_pool(name="ps_tr", bufs=2, space="PSUM") as ps_tr, \
             tc.tile_pool(name="ps_v", bufs=2, space="PSUM") as ps_v, \
             tc.tile_pool(name="ps_f", bufs=2, space="PSUM") as ps_f:
            for blk in range(NBLK):
                xblk = xnatp.tile([128, BLK * D], xdt, tag="xnat")
                src = x[512 * blk:512 * blk + 512, :] \
                    .rearrange("(j p) d -> p j d", p=128)
                dst = xblk[:].rearrange("p (j d) -> p j d", j=BLK)
                if xbf16:
                    nc.gpsimd.dma_start(dst, src)      # cast f32 -> bf16 in DMA
                else:
                    nc.sync.dma_start(dst, src)
                xnat = [xblk[:, D * j:D * (j + 1)] for j in range(BLK)]
                # transposes: xtr[d][:, block] = x_block^T
                for d in range(NDT):
                    ps = ps_tr.tile([128, 512], xdt, tag="tr")
                    for j in range(BLK):
                        nc.tensor.matmul(
                            ps[:, 128 * j:128 * j + 128],
                            xnat[j][:, 128 * d:128 * d + 128],
                            identx[:],
                            start=True, stop=True, is_transpose=True,
                            skip_group_check=True)
                    if d % 2 == 0:
                        nc.scalar.copy(xtr[d][:, 512 * blk:512 * blk + 512], ps[:])
                    else:
                        nc.vector.tensor_copy(xtr[d][:, 512 * blk:512 * blk + 512], ps[:])
                # P1: v^T for block, accumulate over d
                psv = ps_v.tile([16, 512], f32, tag="v")
                for d in range(NDT):
                    nc.tensor.matmul(
                        psv[:], wv_d[d], xtr[d][:, 512 * blk:512 * blk + 512],
                        start=(d == 0), stop=(d == NDT - 1))
                vts = sbA.tile([16, 512], xdt, tag="vts")
                svp = sbA.tile([16, 1], f32, name="svp", tag=f"svp{blk}", bufs=1)
                nc.scalar.activation(vts[:], psv[:], Copy, accum_out=svp[:])
                svps.append(svp)
                # fold-transpose to v natural [128, 16] per l-tile
                psf = ps_f.tile([128, BLK * 16], xdt, tag="vf")
                for j in range(BLK):
                    nc.tensor.matmul(
                        psf[:, 16 * j:16 * j + 16],
                        vts[:, 128 * j:128 * j + 128],
                        identx[0:16, 0:16],
                        start=True, stop=True, is_transpose=True,
                        skip_group_check=True)
                vsum = vnp.tile([128, BLK * 16], xdt, tag="vsum")
                vs3 = vsum[:].rearrange("p (j h) -> p j h", j=BLK)
                pf3 = psf[:].rearrange("p (j h) -> p j h", j=BLK)
                nc.vector.tensor_add(
                    vs3, pf3, bvtb[:].unsqueeze(1).broadcast_to([128, BLK, H]))
                # P2: xv^T accumulation
                for j in range(BLK):
                    nxv[0] += 1
                    for c in range(2):
                        nc.tensor.matmul(
                            xv_ps[c][:], vsum[:, 16 * j:16 * j + 16],
                            xnat[j][:, 512 * c:512 * c + 512],
                            start=(nxv[0] == 1), stop=(nxv[0] == NLT))

        # xv drain + sv
        xvt = pers.tile([H, D], f32, tag="xvt")
        for c in range(2):
            nc.scalar.copy(xvt[:, 512 * c:512 * c + 512], xv_ps[c][:])
        sv = pers.tile([H, 1], f32, tag="sv")
        nc.vector.tensor_add(sv[:], svps[0][:], svps[1][:])
        for b in range(2, NBLK):
            nc.vector.tensor_add(sv[:], sv[:], svps[b][:])
        bvL = pers.tile([H, 1], f32, tag="bvL")
        nc.scalar.mul(bvL[:], cstt[0:16, 160:161], float(L))
        nc.vector.tensor_add(sv[:], sv[:], bvL[:])

        # ---------------- PHASE B ----------------
        with tc.tile_pool(name="sbB", bufs=2) as sbB, \
             tc.tile_pool(name="prep", bufs=1) as prep:
            bdmt = []
            for k in range(NDT):
                m = prep.tile([128, H], f32, name=f"bdm{k}", tag=f"bdm{k}")
                nc.vector.memset(m[:], 0.0)
                nc.vector.memset(m[0:64, 2 * k:2 * k + 1], 1.0)
                nc.vector.memset(m[64:128, 2 * k + 1:2 * k + 2], 1.0)
                bdmt.append(m)
            # xv natural tiles [128, 16] f32r per d-tile
            xvp = []
            with tc.tile_pool(name="ps_m1", bufs=2, space="PSUM") as ps_m1:
                for k in range(NDT):
                    psm = ps_m1.tile([128, 16], f32, tag="m1")
                    nc.tensor.matmul(
                        psm[:], xvt[:, 128 * k:128 * k + 128], ident[0:16, 0:16],
                        start=True, stop=True, is_transpose=True,
                        skip_group_check=True)
                    p = prep.tile([128, 16], f32r, name=f"xvp{k}", tag=f"xvp{k}")
                    nc.scalar.copy(p[:], psm[:])
                    xvp.append(p)
            # s3: ktv_full^T = xv^T Wk
            with tc.tile_pool(name="ps_s3", bufs=1, space="PSUM") as ps_s3, \
                 tc.tile_pool(name="wkjit", bufs=3) as wkjit:
                ps3 = [ps_s3.tile([16, 512], f32, name=f"s3{c}", tag=f"s3{c}")
                       for c in range(2)]
                for k in range(NDT):
                    if xbf16:
                        wkk = wkt[k]
                    else:
                        wkk = wkjit.tile([128, D], f32r, name="wkt", tag="wkt")
                        nc.sync.dma_start(wkk[:], wk[128 * k:128 * k + 128, :])
                    for c in range(2):
                        nc.tensor.matmul(
                            ps3[c][:], xvp[k][:], wkk[:, 512 * c:512 * c + 512],
                            start=(k == 0), stop=(k == NDT - 1))
                ktvt = sbB.tile([H, D], f32, tag="ktvt", bufs=1)
                nc.scalar.activation(ktvt[:], bkt[:], Copy, scale=sv[:])
                for c in range(2):
                    sl = ktvt[:, 512 * c:512 * c + 512]
                    nc.vector.tensor_add(sl, sl, ps3[c][:])
            # ktv_bd natural tiles + c accumulation
            ktvp = []
            with tc.tile_pool(name="ps_m2", bufs=2, space="PSUM") as ps_m2, \
                 tc.tile_pool(name="ps_c", bufs=1, space="PSUM") as ps_c:
                for k in range(NDT):
                    psm = ps_m2.tile([128, 16], f32, tag="m2")
                    nc.tensor.matmul(
                        psm[:], ktvt[:, 128 * k:128 * k + 128], ident[0:16, 0:16],
                        start=True, stop=True, is_transpose=True,
                        skip_group_check=True)
                    nc.vector.tensor_mul(psm[:], psm[:], bdmt[k][:])
                    p = prep.tile([128, 16], f32r, name=f"ktvp{k}", tag=f"ktvp{k}")
                    nc.scalar.copy(p[:], psm[:])
                    ktvp.append(p)
                psc = ps_c.tile([16, 2], f32, tag="c", bufs=1)
                for k in range(NDT):
                    nc.tensor.matmul(
                        psc[:], ktvp[k][:], bqc[:, 2 * k:2 * k + 2],
                        start=(k == 0), stop=(k == NDT - 1))
                cdiv8 = pers.tile([H, 1], f32, tag="cdiv8")
                nc.scalar.mul(cdiv8[:], psc[:, 0:1], 0.125)
            # s4: U^T accumulation with JIT-transposed Wq
            with tc.tile_pool(name="ps_s4", bufs=1, space="PSUM") as ps_s4, \
                 tc.tile_pool(name="wqcp", bufs=3) as wqcp, \
                 tc.tile_pool(name="wqtp", bufs=2) as wqtp, \
                 tc.tile_pool(name="ps_q", bufs=2, space="PSUM") as ps_q:
                ps4 = [ps_s4.tile([16, 512], f32, name=f"s4{c}", tag=f"s4{c}")
                       for c in range(2)]
                for b in range(NDT):
                    wc = wqcp.tile([128, D], f32r, name="wqc", tag="wqc")
                    nc.sync.dma_start(
                        wc[:].rearrange("p (k j) -> p k j", k=NDT),
                        wq[:, 128 * b:128 * b + 128]
                        .rearrange("(k p) j -> p k j", p=128))
                    wt = wqtp.tile([128, D], f32r, name="wqt", tag="wqt")
                    for half in range(2):
                        psq = ps_q.tile([128, 512], f32, name="psq", tag="q")
                        for kk in range(4):
                            k = 4 * half + kk
                            nc.tensor.matmul(
                                psq[:, 128 * kk:128 * kk + 128],
                                wc[:, 128 * k:128 * k + 128].bitcast(f32), ident[:],
                                start=True, stop=True, is_transpose=True,
                                skip_group_check=True)
                        if half == 0:
                            nc.scalar.copy(wt[:, 0:512], psq[:])
                        else:
                            nc.vector.tensor_copy(wt[:, 512:1024], psq[:])
                    for c in range(2):
                        nc.tensor.matmul(
                            ps4[c][:], ktvp[b][:], wt[:, 512 * c:512 * c + 512],
                            start=(b == 0), stop=(b == NDT - 1))
                ut = sbB.tile([H, D], f32, name="ut", tag="ut", bufs=1)
                for c in range(2):
                    nc.scalar.copy(ut[:, 512 * c:512 * c + 512], ps4[c][:])
            # U natural tiles [128, 16] per d-tile
            upr = []
            with tc.tile_pool(name="ps_m3", bufs=2, space="PSUM") as ps_m3:
                for d in range(NDT):
                    psm = ps_m3.tile([128, 16], f32, tag="m3")
                    nc.tensor.matmul(
                        psm[:], ut[:, 128 * d:128 * d + 128], ident[0:16, 0:16],
                        start=True, stop=True, is_transpose=True,
                        skip_group_check=True)
                    p = prep.tile([128, 16], xdt, name=f"upr{d}", tag=f"upr{d}")
                    nc.vector.tensor_copy(p[:], psm[:])
                    upr.append(p)
            # P5: z^T chunks + sigmoid + store
            with tc.tile_pool(name="ps_5", bufs=2, space="PSUM") as ps_5:
                for ch in range(8):
                    ps5 = ps_5.tile([16, 512], f32, tag="s5")
                    for d in range(NDT):
                        nc.tensor.matmul(
                            ps5[:], upr[d][:], xtr[d][:, 512 * ch:512 * ch + 512],
                            start=(d == 0), stop=(d == NDT - 1))
                    sg = sbB.tile([H, 512], f32, name="sg", tag="sg")
                    nc.scalar.activation(sg[:], ps5[:], Sigmoid,
                                         bias=cdiv8[:], scale=0.125)
                    nc.scalar.dma_start(out[:, 512 * ch:512 * ch + 512], sg[:])
    return nc


def ref_numpy(x, wq, wk, wv, bq, bk, bv):
    """f64 reference of the decomposed math for per-stage validation."""
    x64 = x.astype(np.float64)
    v = x64 @ wv.astype(np.float64) + bv.astype(np.float64)   # [L, H]
    xv = x64.T @ v                                            # [D, H]
    ktvfull = wk.astype(np.float64).T @ xv                    # [D, H]
    sv = v.sum(axis=0)                                        # [H]
    ktvfull = ktvfull + np.outer(bk.astype(np.float64), sv)
    bd = np.zeros((D, H))
    for h in range(H):
        bd[64 * h:64 * h + 64, h] = 1.0
    ktvbd = ktvfull * bd
    u = wq.astype(np.float64) @ ktvbd                         # [D, H]
    c = bq.astype(np.float64) @ ktvbd                         # [H]
    z = (x64 @ u + c) / 8.0                                   # [L, H]
    p = 1.0 / (1.0 + np.exp(-z))
    return dict(v=v, xvt=xv.T, ktvbdt=ktvbd.T, ut=u.T, c=c / 8.0, out=p.T)


B = 8
_cache = {}

def _get_nc(xbf16=True):
    key = ("nc", xbf16)
    if key not in _cache:
        _cache[key] = build(xbf16=xbf16)
    return _cache[key]


def _make_cst(Wq, Wk, Wv, bq, bk, bv):
    cstm = np.zeros((128, 176), dtype=np.float32)
    for k in range(NDT):
        cstm[:, 16 * k:16 * k + 16] = Wv[128 * k:128 * k + 128, :]
    bqt = bq.reshape(8, 128).T
    cstm[:, 128:144:2] = bqt
    cstm[:, 129:144:2] = bqt
    cstm[:, 144:160] = np.broadcast_to(bv[None, :], (128, 16))
    cstm[0:16, 160] = bv
    return np.ascontiguousarray(cstm)


def kernel(x, mask, Wq, bq, Wk, bk, Wv, bv, xbf16=True):
    from concourse.bass_utils import run_bass_kernel_spmd
    x = np.asarray(x, dtype=np.float32)
    mask_f = np.asarray(mask).astype(np.float32)
    Wq = np.ascontiguousarray(np.asarray(Wq, dtype=np.float32))
    Wk = np.ascontiguousarray(np.asarray(Wk, dtype=np.float32))
    Wv = np.ascontiguousarray(np.asarray(Wv, dtype=np.float32))
    bq = np.asarray(bq, dtype=np.float32)
    bk = np.asarray(bk, dtype=np.float32)
    bv = np.asarray(bv, dtype=np.float32)
    nc = _get_nc(xbf16)
    cstm = _make_cst(Wq, Wk, Wv, bq, bk, bv)
    bkb_ = np.ascontiguousarray(np.broadcast_to(bk[None, :], (H, D)))
    in_maps = []
    for b in range(B):
        in_maps.append({
            "x": np.ascontiguousarray(x[b]),
            "wq": Wq, "wk": Wk,
            "cst": cstm, "bkb": bkb_,
        })
    res = run_bass_kernel_spmd(nc, in_maps, core_ids=list(range(B)))
    out = np.stack([res.results[b]["out"] for b in range(B)], axis=0)
    out = out * mask_f[:, None, :]
    return out.astype(np.float32)
